# revision 57
# baseline (speedup 1.0000x reference)
"""DeepseekV3 MLA attention forward on 8 Trainium2 NeuronCores.

Sharding: core c -> batch c//4, head group c%4 (4 of 16 heads).
Per-core Bass kernel computes the full MLA forward for its (batch, heads)
slice; host sums the 4 partial wo-projections per batch.

All rope interleaving / head packing / softmax scale are folded into
host-side weight preprocessing. Matmuls run in bf16 with f32 PSUM
accumulation. Attention uses S^T = K^T Q chunks (keys on partitions,
queries on free dim), exp without max subtraction (scores are bounded),
multiplicative causal mask tiles, and a ones-augmented V so the softmax
denominator falls out of the PV matmul.
"""

import math

import numpy as np
import ml_dtypes

import concourse.bass as bass
import concourse.tile as tile
import concourse.mybir as mybir
from concourse import bacc
from concourse.bass_utils import run_bass_kernel_spmd

BF16 = mybir.dt.bfloat16
F32 = mybir.dt.float32
AF = mybir.ActivationFunctionType

# ---- model config (hardcoded to match the problem spec) ----
HIDDEN = 2048
N_HEADS = 16
Q_LORA = 1536
KV_LORA = 512
NOPE = 128
ROPE = 64
VHD = 128
QHD = NOPE + ROPE  # 192
BASE = 10000.0
SCALE = 40.0
ORIG_MAX = 4096
BETA_FAST = 32
BETA_SLOW = 1
EPS = 1e-6
B = 2
S = 2048

N_CORES = 8
HL = 4          # heads per core
P = 128
TT = S // P     # 16 token tiles
QC = S // 512   # 4 query chunks of 512
KT = S // P     # 16 key tiles

_m = 0.1 * math.log(SCALE) + 1.0
SOFT_SCALE = (QHD ** -0.5) * _m * _m


def _yarn_cos_sin(seq_len):
    dim = ROPE
    ar = np.arange(0, dim, 2, dtype=np.float32)
    freq_extra = 1.0 / BASE ** (ar / dim)
    freq_inter = 1.0 / (SCALE * BASE ** (ar / dim))
    low = math.floor(dim * math.log(ORIG_MAX / (BETA_FAST * 2 * math.pi)) / (2 * math.log(BASE)))
    high = math.ceil(dim * math.log(ORIG_MAX / (BETA_SLOW * 2 * math.pi)) / (2 * math.log(BASE)))
    low, high = max(low, 0), min(high, dim - 1)
    denom = (high - low) if high != low else 0.001
    ramp = np.clip((np.arange(dim // 2, dtype=np.float32) - low) / denom, 0.0, 1.0)
    inv_freq_mask = 1.0 - ramp
    inv_freq = freq_inter * (1.0 - inv_freq_mask) + freq_extra * inv_freq_mask
    t = np.arange(seq_len, dtype=np.float32)
    freqs = np.outer(t, inv_freq)
    emb = np.concatenate([freqs, freqs], axis=-1)
    # mscale ratio is 1.0 for this config
    return np.cos(emb).astype(np.float32), np.sin(emb).astype(np.float32)


_PERM64 = np.concatenate([np.arange(0, 64, 2), np.arange(1, 64, 2)])


def _bf16(x):
    return np.ascontiguousarray(x.astype(ml_dtypes.bfloat16))


def _emit_stage_c(nc, tc, psS, kn_sb, q_nope, qpe_rot, krotT, vaug, attnT, masks):
    with (
        tc.tile_pool(name="pt", bufs=16) as ptp,
        tc.tile_pool(name="workC", bufs=4) as workC,
        tc.tile_pool(name="maskp", bufs=1) as maskp,
        tc.tile_pool(name="psO", bufs=4, space="PSUM") as psO,
    ):
        mask_sb = maskp.tile([P, 896], BF16, tag="mask_sb")
        nc.sync.dma_start(mask_sb[:], masks[:])
        for qc in range(QC):
            for h in range(HL):
                nkt = 4 * qc + 4
                pts = []
                offs = []
                for kt in range(nkt):
                    diag = (kt // 4 == qc)
                    off = (kt % 4) * P if diag else 0
                    w = 512 - off
                    ss = psS.tile([P, 512], F32, tag="ss")
                    nc.tensor.matmul(
                        ss[:, 0:w], kn_sb[:, h, kt * P:(kt + 1) * P],
                        q_nope[:, h, qc * 512 + off:(qc + 1) * 512],
                        start=True, stop=False)
                    nc.tensor.matmul(
                        ss[:, 0:w], krotT[:, kt * P:(kt + 1) * P],
                        qpe_rot[:, h, qc * 512 + off:(qc + 1) * 512],
                        start=False, stop=True)
                    pt_t = ptp.tile([P, 512], BF16, tag="pt")
                    nc.scalar.activation(pt_t[:, 0:w], ss[:, 0:w], AF.Exp,
                                         scale=SOFT_SCALE)
                    if diag:
                        nc.vector.tensor_mul(pt_t[:, 0:P], pt_t[:, 0:P],
                                             mask_sb[:, 384:384 + P])
                    pts.append(pt_t)
                    offs.append(off)
                for qsub in range(4):
                    qt = 4 * qc + qsub
                    po = psO.tile([P, VHD + 1], F32, tag="po")
                    for kt in range(qt + 1):
                        o = qsub * P - offs[kt]
                        nc.tensor.matmul(
                            po[:], pts[kt][:, o:o + P],
                            vaug[:, h, kt, 0:VHD + 1],
                            start=(kt == 0), stop=(kt == qt))
                    rd = workC.tile([P, 1], F32, tag="rd")
                    nc.vector.reciprocal(rd[:], po[:, VHD:VHD + 1])
                    at = workC.tile([P, VHD], BF16, tag="at")
                    nc.vector.tensor_scalar_mul(at[:], po[:, 0:VHD], rd[:])
                    nc.sync.dma_start_transpose(
                        attnT[:, h, qt * P:(qt + 1) * P], at[:])


def _emit_stage_b(nc, tc, stages, pools, tensors):
    wBQ, workB, psB, psBq = pools
    (wqb_sb, wkvb_sb, qnT, cnT, costok, sintok,
     kn_sb, vaug, q_nope, qpe_rot, wqbT) = tensors
    KQ = Q_LORA // P
    KKV = KV_LORA // P
    for k in range(KQ):
        nc.sync.dma_start(wqb_sb[:, k], wqbT[k * P:(k + 1) * P, :])

    # ---- B-KV first (wkvb preloaded; wqb loads overlap) ----
    for tcks in range(4):
        for ft in range(2 * HL):
            ps = psB.tile([P, 512], F32, tag="up")
            for k in range(KKV):
                nc.tensor.matmul(
                    ps[:], wkvb_sb[:, k, ft * P:(ft + 1) * P],
                    cnT[:, k, tcks * 512:(tcks + 1) * 512],
                    start=(k == 0), stop=(k == KKV - 1))
            if ft < HL:
                nc.scalar.copy(kn_sb[:, ft, tcks * 512:(tcks + 1) * 512], ps[:])
            else:
                h = ft - HL
                vs = workB.tile([P, 512], BF16, tag="vs")
                nc.scalar.copy(vs[:], ps[:])
                for lk in range(4):
                    kt = tcks * 4 + lk
                    nc.sync.dma_start_transpose(
                        vaug[:, h, kt, 0:VHD], vs[:, lk * P:(lk + 1) * P])

    # ---- B-Q: nope (tc-outer) then pe per token tile ----
    for tcks in range(4):
        for ft in range(HL):
            ps = psB.tile([P, 512], F32, tag="up")
            for k in range(KQ):
                nc.tensor.matmul(
                    ps[:], wqb_sb[:, k, ft * P:(ft + 1) * P],
                    qnT[:, k, tcks * 512:(tcks + 1) * 512],
                    start=(k == 0), stop=(k == KQ - 1))
            nc.scalar.copy(q_nope[:, ft, tcks * 512:(tcks + 1) * 512], ps[:])
        for tsub in range(4):
            t = tcks * 4 + tsub
            qpe_ps = psBq.tile([P, HL * 64], F32, tag="qpe_ps")
            for k in range(KQ):
                nc.tensor.matmul(
                    qpe_ps[:], qnT[:, k, t * P:(t + 1) * P],
                    wqb_sb[:, k, HL * P:HL * P + HL * 64],
                    start=(k == 0), stop=(k == KQ - 1))
            qr = workB.tile([P, HL, 2 * 64], BF16, tag="qr")
            qtmp = workB.tile([P, 64], BF16, tag="qtmp")
            nc.vector.memset(qr[:, :, 64:], 0.0)
            for h in range(HL):
                hs0 = h * 64
                nc.vector.tensor_mul(qr[:, h, 0:64], qpe_ps[:, hs0:hs0 + 64],
                                     costok[:, t])
                nc.vector.tensor_mul(qtmp[:, 0:32], qpe_ps[:, hs0 + 32:hs0 + 64],
                                     sintok[:, t, 0:32])
                nc.vector.tensor_mul(qtmp[:, 32:64], qpe_ps[:, hs0:hs0 + 32],
                                     sintok[:, t, 32:64])
                nc.vector.tensor_add(qr[:, h, 0:64], qr[:, h, 0:64], qtmp[:])
                qsc = workB.tile([P, P], BF16, tag="qsc")
                nc.sync.dma_start_transpose(qsc[:], qr[:, h])
                nc.vector.tensor_copy(qpe_rot[:, h, t * P:(t + 1) * P], qsc[0:64])


def _build_nc(stages="ABCD"):
    nc = bacc.Bacc("TRN2", target_bir_lowering=False, debug=False, num_devices=N_CORES)

    hT = nc.declare_dram_parameter("hT", [HIDDEN, S], BF16, isOutput=False)
    wqaT = nc.declare_dram_parameter("wqaT", [HIDDEN, Q_LORA], BF16, isOutput=False)
    wkvaT = nc.declare_dram_parameter("wkvaT", [HIDDEN, KV_LORA + ROPE], BF16, isOutput=False)
    wqbT = nc.declare_dram_parameter("wqbT", [Q_LORA, HL * QHD], BF16, isOutput=False)
    wkvbT = nc.declare_dram_parameter("wkvbT", [KV_LORA, HL * (NOPE + VHD)], BF16, isOutput=False)
    woT = nc.declare_dram_parameter("woT", [HL * VHD, HIDDEN], BF16, isOutput=False)
    cos_tm = nc.declare_dram_parameter("cos_tm", [S, ROPE], BF16, isOutput=False)
    sin_tm_s = nc.declare_dram_parameter("sin_tm_s", [S, ROPE], BF16, isOutput=False)
    masks = nc.declare_dram_parameter("masks", [P, 896], BF16, isOutput=False)
    outT = nc.declare_dram_parameter("outT", [HIDDEN, S], F32, isOutput=True)

    KH = HIDDEN // P    # 16
    KQ = Q_LORA // P    # 12
    KKV = KV_LORA // P  # 4

    with tile.TileContext(nc) as tc:
        with tc.tile_pool(name="glob", bufs=1) as pp:
            qnT = pp.tile([P, KQ, S], BF16, tag="qnT")
            cnT = pp.tile([P, KKV, S], BF16, tag="cnT")
            krotT = pp.tile([64, S], BF16, tag="krotT")
            attnT = pp.tile([P, HL, S], BF16, tag="attnT")
            costok = pp.tile([P, TT, ROPE], BF16, tag="costok")
            sintok = pp.tile([P, TT, ROPE], BF16, tag="sintok")
            eps_sb = pp.tile([P, 1], F32, tag="eps")
            wkvb_sb = pp.tile([P, KKV, HL * (NOPE + VHD)], BF16, tag="wkvb")
            nc.vector.memset(eps_sb[:], EPS)

            # ====== Stage A ==================================================
            if "A" in stages:
                with (
                    tc.tile_pool(name="wA", bufs=1) as wA,
                    tc.tile_pool(name="htp", bufs=4) as htp,
                    tc.tile_pool(name="workA", bufs=3) as workA,
                    tc.tile_pool(name="scr", bufs=3) as scr,
                    tc.tile_pool(name="psA", bufs=2, space="PSUM") as psA,
                    tc.tile_pool(name="psA2", bufs=1, space="PSUM") as psA2,
                ):
                    # prefetch first two token tiles before the weight stream
                    ht_pre = {}
                    for t in (0, 1, 2, 3):
                        ht_pre[t] = htp.tile([P, KH, P], BF16, tag="ht",
                                             name=f"ht{t}")
                        nc.sync.dma_start(
                            ht_pre[t][:],
                            hT[:, t * P:(t + 1) * P].rearrange("(k p) t -> p k t", p=P))
                    wqa_sb = wA.tile([P, KH, Q_LORA], BF16, tag="wqa")
                    wkva_sb = wA.tile([P, KH, KV_LORA + ROPE], BF16, tag="wkva")
                    for k in range(KH):
                        nc.sync.dma_start(wqa_sb[:, k], wqaT[k * P:(k + 1) * P, :])
                        nc.sync.dma_start(wkva_sb[:, k], wkvaT[k * P:(k + 1) * P, :])
                    for t in range(TT):
                        nc.sync.dma_start(costok[:, t], cos_tm[t * P:(t + 1) * P, :])
                        nc.sync.dma_start(sintok[:, t], sin_tm_s[t * P:(t + 1) * P, :])
                    for k in range(KKV):
                        nc.sync.dma_start(wkvb_sb[:, k], wkvbT[k * P:(k + 1) * P, :])

                    for t in range(TT):
                        if t in ht_pre:
                            ht = ht_pre[t]
                        else:
                            ht = htp.tile([P, KH, P], BF16, tag="ht", name=f"ht{t}")
                            nc.sync.dma_start(
                                ht[:],
                                hT[:, t * P:(t + 1) * P].rearrange("(k p) t -> p k t",
                                                                   p=P))
                        qd3 = psA.tile([P, 3, 512], F32, tag="qd", name=f"qd{t}")
                        ckv = psA2.tile([P, KV_LORA + ROPE], F32, tag="ckv")
                        for k in range(KH):
                            st, sp_ = (k == 0), (k == KH - 1)
                            for fc in range(3):
                                nc.tensor.matmul(
                                    qd3[:, fc], ht[:, k],
                                    wqa_sb[:, k, fc * 512:(fc + 1) * 512],
                                    start=st, stop=sp_)
                            nc.tensor.matmul(
                                ckv[:, :KV_LORA], ht[:, k], wkva_sb[:, k, :KV_LORA],
                                start=st, stop=sp_)
                            nc.tensor.matmul(
                                ckv[:, KV_LORA:], ht[:, k], wkva_sb[:, k, KV_LORA:],
                                start=st, stop=sp_)

                        sq = scr.tile([P, 512], BF16, tag="sq")
                        parts = scr.tile([P, 4], F32, tag="parts")
                        for fc in range(3):
                            nc.scalar.activation(sq[:], qd3[:, fc], AF.Square,
                                                 accum_out=parts[:, fc:fc + 1])
                        ssum = scr.tile([P, 2], F32, tag="ssum")
                        nc.vector.reduce_sum(ssum[:, 0:1], parts[:, 0:3],
                                             axis=mybir.AxisListType.X)
                        nc.scalar.activation(ssum[:, 1:2], ssum[:, 0:1], AF.Sqrt,
                                             scale=1.0 / Q_LORA, bias=eps_sb[:])
                        rq = scr.tile([P, 1], F32, tag="rq")
                        nc.vector.reciprocal(rq[:], ssum[:, 1:2])
                        qn = workA.tile([P, 3, 512], BF16, tag="qn")
                        for fc in range(3):
                            nc.vector.tensor_scalar_mul(qn[:, fc], qd3[:, fc], rq[:])

                        sq2 = scr.tile([P, 512], BF16, tag="sq2")
                        parts2 = scr.tile([P, 2], F32, tag="parts2")
                        nc.scalar.activation(sq2[:], ckv[:, :KV_LORA], AF.Square,
                                             accum_out=parts2[:, 0:1])
                        nc.scalar.activation(parts2[:, 1:2], parts2[:, 0:1], AF.Sqrt,
                                             scale=1.0 / KV_LORA, bias=eps_sb[:])
                        rkv = scr.tile([P, 1], F32, tag="rkv")
                        nc.vector.reciprocal(rkv[:], parts2[:, 1:2])
                        cn = workA.tile([P, 512], BF16, tag="cn")
                        nc.vector.tensor_scalar_mul(cn[:], ckv[:, :KV_LORA], rkv[:])

                        kr = workA.tile([P, 2 * ROPE], BF16, tag="kr")
                        tmp = scr.tile([P, ROPE], BF16, tag="tmpr")
                        nc.vector.memset(kr[:, ROPE:], 0.0)
                        nc.vector.tensor_mul(kr[:, 0:ROPE], ckv[:, KV_LORA:],
                                             costok[:, t])
                        nc.vector.tensor_mul(tmp[:, 0:32], ckv[:, KV_LORA + 32:],
                                             sintok[:, t, 0:32])
                        nc.vector.tensor_mul(tmp[:, 32:64],
                                             ckv[:, KV_LORA:KV_LORA + 32],
                                             sintok[:, t, 32:64])
                        nc.vector.tensor_add(kr[:, 0:ROPE], kr[:, 0:ROPE], tmp[:])

                        for f in range(KQ):
                            nc.sync.dma_start_transpose(
                                qnT[:, f, t * P:(t + 1) * P],
                                qn[:, f // 4, (f % 4) * P:((f % 4) + 1) * P])
                        for f in range(KKV):
                            nc.sync.dma_start_transpose(
                                cnT[:, f, t * P:(t + 1) * P],
                                cn[:, f * P:(f + 1) * P])
                        ksc = scr.tile([P, P], BF16, tag="ksc")
                        nc.sync.dma_start_transpose(ksc[:], kr[:])
                        nc.vector.tensor_copy(krotT[:, t * P:(t + 1) * P], ksc[0:64])

            # ====== Stages B + C ============================================
            with tc.tile_pool(name="outs", bufs=1) as outs:
                kn_sb = outs.tile([P, HL, S], BF16, tag="kn_sb")
                vaug = outs.tile([P, HL, KT, VHD + 16], BF16, tag="vaug")
                q_nope = outs.tile([P, HL, S], BF16, tag="q_nope")
                qpe_rot = outs.tile([64, HL, S], BF16, tag="qpe_rot")
                nc.vector.memset(vaug[:, :, :, VHD], 1.0)

                psS = tc.alloc_tile_pool(name="psS", bufs=3, space="PSUM")
                with (
                    tc.tile_pool(name="wBQ", bufs=1) as wBQ,
                    tc.tile_pool(name="workB", bufs=2) as workB,
                    tc.tile_pool(name="psB", bufs=3, space="PSUM") as psB,
                    tc.tile_pool(name="psBq", bufs=2, space="PSUM") as psBq,
                ):
                    wqb_sb = wBQ.tile([P, KQ, HL * QHD], BF16, tag="wqb")
                    if "B" in stages:
                        _emit_stage_b(
                            nc, tc, stages, (wBQ, workB, psB, psBq),
                            (wqb_sb, wkvb_sb, qnT, cnT, costok, sintok,
                             kn_sb, vaug, q_nope, qpe_rot, wqbT))

                with (
                    tc.tile_pool(name="wD", bufs=1) as wD,
                    tc.tile_pool(name="workD", bufs=4) as workD,
                ):
                    wo_sb = wD.tile([P, HL, HIDDEN], BF16, tag="wo")
                    if "D" in stages:
                        for k in range(HL):
                            nc.sync.dma_start(wo_sb[:, k], woT[k * P:(k + 1) * P, :])
                    if "C" in stages:
                        _emit_stage_c(nc, tc, psS, kn_sb, q_nope, qpe_rot, krotT,
                                      vaug, attnT, masks)
                    if "D" in stages:
                        with tc.tile_pool(name="psD", bufs=4, space="PSUM") as psD:
                            for tcks in range(4):
                                for ot in range(HIDDEN // P):
                                    ps = psD.tile([P, 512], F32, tag="wops")
                                    for k in range(HL):
                                        nc.tensor.matmul(
                                            ps[:], wo_sb[:, k, ot * P:(ot + 1) * P],
                                            attnT[:, k, tcks * 512:(tcks + 1) * 512],
                                            start=(k == 0), stop=(k == HL - 1))
                                    ob = workD.tile([P, 512], F32, tag="ob")
                                    nc.scalar.copy(ob[:], ps[:])
                                    nc.sync.dma_start(
                                        outT[ot * P:(ot + 1) * P,
                                             tcks * 512:(tcks + 1) * 512], ob[:])
                psS.release()

    nc.compile()
    return nc


def _get_nc():
    if "nc" not in _NC_CACHE:
        _NC_CACHE["nc"] = _build_nc()
    return _NC_CACHE["nc"]


def kernel(hidden_states, position_ids, wq_a, q_a_ln_w, wq_b, wkv_a, kv_a_ln_w,
           wkv_b, wo):
    hidden_states = np.asarray(hidden_states, dtype=np.float32)
    position_ids = np.asarray(position_ids)
    wq_a = np.asarray(wq_a, dtype=np.float32)
    wq_b = np.asarray(wq_b, dtype=np.float32)
    wkv_a = np.asarray(wkv_a, dtype=np.float32)
    wkv_b = np.asarray(wkv_b, dtype=np.float32)
    wo = np.asarray(wo, dtype=np.float32)
    # fold RMSNorm elementwise weights into the up-projections (exact)
    wq_b = wq_b * np.asarray(q_a_ln_w, dtype=np.float32)[None, :]
    wkv_b = wkv_b * np.asarray(kv_a_ln_w, dtype=np.float32)[None, :]
    assert hidden_states.shape == (B, S, HIDDEN)

    cos_t, sin_t = _yarn_cos_sin(S)

    # --- weight preprocessing (shared across cores in each batch group) ---
    # wq_b rows permuted: per head [nope(128); pe perm64(64)], heads packed as
    # [h0..h3 nope][pe pairs at 64-row offsets]
    wqbT_groups = []
    wkvbT_groups = []
    woT_groups = []
    for g in range(4):
        heads = range(4 * g, 4 * g + 4)
        rows = []
        for h in heads:
            rows.append(np.arange(h * QHD, h * QHD + NOPE))
        pe_rows = []
        for h in heads:
            pe_rows.append(h * QHD + NOPE + _PERM64)
        rows = np.concatenate(rows + pe_rows)
        wqbT_groups.append(_bf16(wq_b[rows].T))

        rows = []
        for h in heads:
            rows.append(np.arange(h * (NOPE + VHD), h * (NOPE + VHD) + NOPE))
        for h in heads:
            rows.append(np.arange(h * (NOPE + VHD) + NOPE, (h + 1) * (NOPE + VHD)))
        rows = np.concatenate(rows)
        wkvbT_groups.append(_bf16(wkv_b[rows].T))

        cols = np.concatenate([np.arange(h * VHD, (h + 1) * VHD) for h in heads])
        woT_groups.append(_bf16(wo[:, cols].T))

    wqaT = _bf16(wq_a.T)
    wkva_perm = wkv_a.copy()
    wkva_perm[KV_LORA:] = wkv_a[KV_LORA + _PERM64]
    wkvaT = _bf16(wkva_perm.T)

    x_idx = np.arange(896)[None, :]
    p_idx = np.arange(P)[:, None]
    masks = _bf16((x_idx >= 384 + p_idx).astype(np.float32))

    # --- per-batch rope tables ---
    batch_tabs = []
    for beta in range(B):
        pos = position_ids[beta].astype(np.int64)
        cg = cos_t[pos]          # [S, 64]
        sg = sin_t[pos]
        sin_s = np.concatenate([-sg[:, :32], sg[:, 32:]], axis=1)
        batch_tabs.append((
            _bf16(cg), _bf16(sin_s),
            _bf16(hidden_states[beta].T),
        ))

    in_maps = []
    for c in range(N_CORES):
        beta, g = c // 4, c % 4
        cg, sin_s, hT = batch_tabs[beta]
        in_maps.append({
            "hT": hT,
            "wqaT": wqaT,
            "wkvaT": wkvaT,
            "wqbT": wqbT_groups[g],
            "wkvbT": wkvbT_groups[g],
            "woT": woT_groups[g],
            "cos_tm": cg,
            "sin_tm_s": sin_s,
            "masks": masks,
        })

    nc = _get_nc()
    global _LAST_RES, _LAST_IN_MAPS
    _LAST_IN_MAPS = in_maps
    res = run_bass_kernel_spmd(nc, in_maps, core_ids=list(range(N_CORES)))
    _LAST_RES = res

    out = np.zeros((B, S, HIDDEN), dtype=np.float32)
    for c in range(N_CORES):
        out[c // 4] += res.results[c]["outT"].T
    return out
